# revision 1
# baseline (speedup 1.0000x reference)
"""GQA attention kernel for 8 Trainium2 NeuronCores.

Sharding: core c -> (b = c // 4, kv-group gk = c % 4).
Each core computes, for its batch b and its kv head gk (which owns the 4
contiguous q-heads gk*4..gk*4+3):
    q/k/v projections, attention, and a partial out-projection
    out_partial[b] = o_heads @ Wo[:, gk*512:(gk+1)*512].T
Host sums the 4 partials per batch.

All matmuls in bf16 (fp32 PSUM accumulation). Softmax without max
subtraction (scores are bounded ~|4.5| at this problem's weight scale);
row sums come free from a ones-column appended to V; normalization is
applied to the 128-wide per-head output ahead of the out projection.

Layout (per core), everything E/K-major for the PE:
  xT  [E, N]   = x[b].T          kT [128d, N]    scoresT [s, n] chunks
  wq  [E, 512] = Wq rows.T       qT [128, 4g, N]
  wk  [E, 128] = Wk rows.T       v  [128, 16st, 130] (col 128 = ones)
  wv  [E, 128]                   oT [128, 4g, N]
  wo  [512, E] = Wo cols.T       out [N, E] f32 partial
"""

import sys

sys.path.insert(0, "/opt/trn_rl_repo")

import numpy as np
import ml_dtypes

import concourse.bass as bass
import concourse.mybir as mybir
import concourse.tile as tile
from concourse import bacc
from concourse.bass_utils import run_bass_kernel_spmd
from concourse.masks import make_identity

BF16 = mybir.dt.bfloat16
F32 = mybir.dt.float32
bf16 = ml_dtypes.bfloat16

B, N, E = 2, 2048, 2048
H, D, G = 16, 128, 4
HKV = H // G
JL = G * D                     # 512 local q-head dims per core
ET = E // 128                  # 16
NT = N // 128                  # 16
CH = N // 512                  # 4
SCALE = 1.0 / float(np.sqrt(D))

_cached = {}


def _build(iters=1):
    nc = bacc.Bacc("TRN2", target_bir_lowering=False, debug=False, num_devices=8)

    xT = nc.dram_tensor("xT", [E, N], BF16, kind="ExternalInput")
    wq = nc.dram_tensor("wq", [E, JL], BF16, kind="ExternalInput")
    wk = nc.dram_tensor("wk", [E, D], BF16, kind="ExternalInput")
    wv = nc.dram_tensor("wv", [E, D], BF16, kind="ExternalInput")
    wo = nc.dram_tensor("wo", [JL, E], BF16, kind="ExternalInput")
    out = nc.dram_tensor("out", [N, E], F32, kind="ExternalOutput")

    with tile.TileContext(nc) as tc:
        with (
            tc.tile_pool(name="const", bufs=1) as cpool,
            tc.tile_pool(name="xp", bufs=1) as xpool,
            tc.tile_pool(name="wp", bufs=1) as wpool,
            tc.tile_pool(name="kvp", bufs=1) as kvpool,
            tc.tile_pool(name="qp", bufs=1) as qpool,
            tc.tile_pool(name="pp", bufs=2) as ppool,
            tc.tile_pool(name="op", bufs=4) as opool,
            tc.tile_pool(name="otp", bufs=1) as otpool,
            tc.tile_pool(name="outp", bufs=3) as outpool,
            tc.tile_pool(name="ps1", bufs=2, space="PSUM") as P1,
            tc.tile_pool(name="ps2", bufs=4, space="PSUM") as P2,
        ):
            ident = cpool.tile([128, 128], BF16, tag="ident")
            make_identity(nc, ident[:])

            for _ in range(iters):
                _emit_iter(nc, tc, ident, xpool, wpool, kvpool, qpool, ppool,
                           opool, otpool, outpool, P1, P2,
                           xT, wq, wk, wv, wo, out)

    nc.compile()
    return nc


def _emit_iter(nc, tc, ident, xpool, wpool, kvpool, qpool, ppool, opool,
               otpool, outpool, P1, P2, xT, wq, wk, wv, wo, out):
    x_sb = xpool.tile([128, ET, N], BF16, tag="x")
    wq_sb = wpool.tile([128, ET, JL], BF16, tag="wq")
    wk_sb = wpool.tile([128, ET, D], BF16, tag="wk")
    wv_sb = wpool.tile([128, ET, D], BF16, tag="wv")
    wo_sb = wpool.tile([128, G, E], BF16, tag="wo")
    kT_sb = kvpool.tile([128, N], BF16, tag="kT")
    v_sb = kvpool.tile([128, NT, 130], BF16, tag="v")
    qT_sb = qpool.tile([128, G, N], BF16, tag="qT")
    oT_sb = otpool.tile([128, G, N], BF16, tag="oT")

    # --- input DMAs, in consumption order ---
    # wk / wv: single batched DMA each ([E,D] -> [128, ET, D])
    nc.sync.dma_start(wk_sb[:], wk.rearrange("(a p) d -> p a d", p=128))
    nc.scalar.dma_start(wv_sb[:], wv.rearrange("(a p) d -> p a d", p=128))
    # x: 8 DMAs of 1MB (two e-tiles each)
    xr = xT.rearrange("(a p) n -> p a n", p=128)
    for i in range(8):
        eng = nc.sync if i % 2 == 0 else nc.scalar
        eng.dma_start(x_sb[:, 2 * i:2 * i + 2, :], xr[:, 2 * i:2 * i + 2, :])
    # wq: 2 DMAs, wo: 4 DMAs
    wqr = wq.rearrange("(a p) j -> p a j", p=128)
    for i in range(2):
        nc.gpsimd.dma_start(wq_sb[:, 8 * i:8 * i + 8, :], wqr[:, 8 * i:8 * i + 8, :])
    for jt in range(G):
        nc.gpsimd.dma_start(wo_sb[:, jt, :], wo[jt * 128:(jt + 1) * 128, :])

    nc.vector.memset(v_sb[:, :, 128:129], 1.0)

    # --- phase 1 ---
    # kT: 4 chunk accumulators (2 double-bank P1 tiles) so the PE can trail
    # the x DMAs; v head tiles on P2 meanwhile.
    kp = [P1.tile([128, 1024], F32, tag="mm1024", name=f"kp{_i}") for _i in range(2)]
    kps = [kp[_i // 2][:, (_i % 2) * 512:(_i % 2 + 1) * 512] for _i in range(CH)]
    vps = [P2.tile([128, 130], F32, tag="oc", name=f"vps{_i}") for _i in range(4)]
    for et in range(ET):
        for sc in range(CH):
            nc.tensor.matmul(
                kps[sc], wk_sb[:, et, :], x_sb[:, et, sc * 512:(sc + 1) * 512],
                start=(et == 0), stop=(et == ET - 1),
            )
        for st in range(4):
            nc.tensor.matmul(
                vps[st][:, 0:128], x_sb[:, et, st * 128:(st + 1) * 128],
                wv_sb[:, et, :],
                start=(et == 0), stop=(et == ET - 1),
            )
    for sc in range(CH):
        nc.vector.tensor_copy(kT_sb[:, sc * 512:(sc + 1) * 512], kps[sc])
    for st in range(4):
        nc.vector.tensor_copy(v_sb[:, st, 0:128], vps[st][:, 0:128])

    # remaining v tiles interleaved with paired q groups
    def emit_v(st):
        ps = P2.tile([128, 130], F32, tag="oc")
        for et in range(ET):
            nc.tensor.matmul(
                ps[:, 0:128], x_sb[:, et, st * 128:(st + 1) * 128],
                wv_sb[:, et, :],
                start=(et == 0), stop=(et == ET - 1),
            )
        nc.vector.tensor_copy(v_sb[:, st, 0:128], ps[:, 0:128])

    def emit_q_pair(q0, q1):
        ps = P1.tile([128, 1024], F32, tag="mm1024")
        for half, (g, ncg) in enumerate((q0, q1)):
            sl = ps[:, half * 512:(half + 1) * 512]
            for et in range(ET):
                nc.tensor.matmul(
                    sl, wq_sb[:, et, g * 128:(g + 1) * 128],
                    x_sb[:, et, ncg * 512:(ncg + 1) * 512],
                    start=(et == 0), stop=(et == ET - 1),
                )
            nc.vector.tensor_copy(qT_sb[:, g, ncg * 512:(ncg + 1) * 512], sl)

    qlist = [(g, ncg) for g in range(G) for ncg in range(CH)]
    vq = list(range(4, NT))

    # --- phase 2 + 3, pipelined per chunk of 512 n-columns ---
    # Scores for two s-tiles share one double-bank psum tile so a single
    # (wider, cheaper per element) Exp covers both. o-groups of the
    # previous chunk are interleaved between score pairs to keep the PE
    # busy while ACT digests the exps.
    def emit_score_pair(g, c, p_t, sp):
        ps = P1.tile([128, 1024], F32, tag="mm1024")
        for half in range(2):
            st = 2 * sp + half
            nc.tensor.matmul(
                ps[:, half * 512:(half + 1) * 512],
                kT_sb[:, st * 128:(st + 1) * 128],
                qT_sb[:, g, c * 512:(c + 1) * 512],
                start=True, stop=True,
            )
        nc.scalar.activation(
            p_t[:, 2 * sp * 512:(2 * sp + 2) * 512], ps[:],
            mybir.ActivationFunctionType.Exp, scale=SCALE,
        )

    def emit_o_group(g, c, p_t, t):
        pso = P2.tile([128, 130], F32, tag="oc")
        for st in range(NT):
            nc.tensor.matmul(
                pso[:, 0:129], p_t[:, st * 512 + t * 128: st * 512 + (t + 1) * 128],
                v_sb[:, st, 0:129],
                start=(st == 0), stop=(st == NT - 1),
            )
        rc = opool.tile([128, 1], F32, tag="recip")
        nc.vector.reciprocal(rc[:], pso[:, 128:129])
        o_n = opool.tile([128, 128], BF16, tag="o_n")
        nc.vector.tensor_scalar_mul(o_n[:], pso[:, 0:128], rc[:])
        pst = P2.tile([128, 128], BF16, tag="oc")
        nc.tensor.transpose(pst[:], o_n[:], ident[:])
        nc.vector.tensor_copy(
            oT_sb[:, g, c * 512 + t * 128: c * 512 + (t + 1) * 128], pst[:],
        )

    def emit_out_nt(nt):
        for half in range(2):
            stage = outpool.tile([128, 1024], F32, tag="out")
            ps = P1.tile([128, 1024], F32, tag="mm1024")
            for e2 in range(2):
                ec = half * 2 + e2
                for g in range(G):
                    nc.tensor.matmul(
                        ps[:, e2 * 512:(e2 + 1) * 512],
                        oT_sb[:, g, nt * 128:(nt + 1) * 128],
                        wo_sb[:, g, ec * 512:(ec + 1) * 512],
                        start=(g == 0), stop=(g == G - 1),
                    )
            nc.vector.tensor_copy(stage[:], ps[:])
            eng = nc.sync if (nt + half) % 2 == 0 else nc.scalar
            eng.dma_start(
                out[nt * 128:(nt + 1) * 128, half * 1024:(half + 1) * 1024],
                stage[:],
            )

    def emit_out(c):
        for nt in range(4 * c, 4 * c + 4):
            emit_out_nt(nt)

    for i in range(8):
        emit_q_pair(qlist[2 * i], qlist[2 * i + 1])
        for _ in range(2):
            if vq:
                emit_v(vq.pop(0))

    chunks = [(c, g) for c in range(CH) for g in range(G)]
    prev = None
    for i, (c, g) in enumerate(chunks):
        p_t = ppool.tile([128, NT * 512], BF16, tag="p", name=f"p{i}")
        for sub in range(4):
            emit_score_pair(g, c, p_t, 2 * sub)
            emit_score_pair(g, c, p_t, 2 * sub + 1)
            if prev is not None:
                emit_o_group(prev[0], prev[1], prev[2], sub)
        if i > 0 and i % 4 == 0:
            emit_out(i // 4 - 1)
        prev = (g, c, p_t)
    # tail: interleave the last chunk's o-groups with its out-projection
    for sub in range(4):
        emit_o_group(prev[0], prev[1], prev[2], sub)
        emit_out_nt(4 * (CH - 1) + sub)


def get_nc(iters=1):
    key = ("nc", iters)
    if key not in _cached:
        _cached[key] = _build(iters)
    return _cached[key]


def make_in_maps(x, Wq, Wk, Wv, Wo):
    """Per-core host-side sharding. Core c -> (b=c//4, gk=c%4)."""
    in_maps = []
    xT = [np.ascontiguousarray(x[b].T).astype(bf16) for b in range(B)]
    wq_s = [np.ascontiguousarray(Wq[gk * JL:(gk + 1) * JL, :].T).astype(bf16)
            for gk in range(HKV)]
    wk_s = [np.ascontiguousarray(Wk[gk * D:(gk + 1) * D, :].T).astype(bf16)
            for gk in range(HKV)]
    wv_s = [np.ascontiguousarray(Wv[gk * D:(gk + 1) * D, :].T).astype(bf16)
            for gk in range(HKV)]
    wo_s = [np.ascontiguousarray(Wo[:, gk * JL:(gk + 1) * JL].T).astype(bf16)
            for gk in range(HKV)]
    for c in range(8):
        b, gk = c // 4, c % 4
        in_maps.append({
            "xT": xT[b], "wq": wq_s[gk], "wk": wk_s[gk],
            "wv": wv_s[gk], "wo": wo_s[gk],
        })
    return in_maps


def kernel(x, Wq, Wk, Wv, Wo):
    nc = get_nc()
    in_maps = make_in_maps(x, Wq, Wk, Wv, Wo)
    res = run_bass_kernel_spmd(nc, in_maps, core_ids=list(range(8)))
    out = np.empty((B, N, E), np.float32)
    for b in range(B):
        acc = res.results[b * 4]["out"]
        for gk in range(1, HKV):
            acc = acc + res.results[b * 4 + gk]["out"]
        out[b] = acc
    return out



# revision 2
# speedup vs baseline: 1.0045x; 1.0045x over previous
"""GQA attention kernel for 8 Trainium2 NeuronCores — fp8 DoubleRow edition.

Sharding: core c -> (b = c // 4, kv-group gk = c % 4); host sums the 4
partial out-projections per batch (descaled by 2^-18).

vs the bf16 baseline: the q/k/v projections and the out projection run as
fp8e4m3 DoubleRow matmuls with hi+lo operand splits (3 refined terms per
2 contraction tiles = 0.75x the bf16 PE cycles, ~1e-3 extra error).
Scores and P@V stay bf16 (their operands can't be cheaply split).

Scales (all powers of 2, folded so no extra device ops):
  x*32, Wq/Wk/Wv*4096 -> kT/qT/v carry 2^17; exp scale absorbs 2^-34;
  v ones-column = 2^17 so o comes out unscaled; oT split at *64,
  Wo*4096 -> out partials carry 2^18, descaled on the host.

Layout (per core):
  x_hl  [128, ET, 2, N]  e4m3 (hi,lo)     kT [128, N] bf16 (x 2^17)
  wq/wk/wv_lh [128, ET, 2, .] e4m3 (lo,hi) qT [128, 4g, N] bf16 (x 2^17)
  wo_lh [128, G, 2, E] e4m3 (lo,hi)        v  [128, 16st, 130] bf16
  oT_hl [128, G, 2, N] e4m3 (hi,lo)        out [N, E] f32 partial (x 2^18)
"""

import sys

sys.path.insert(0, "/opt/trn_rl_repo")

import numpy as np
import ml_dtypes

import concourse.bass as bass
import concourse.mybir as mybir
import concourse.tile as tile
from concourse import bacc
from concourse.bass_utils import run_bass_kernel_spmd
from concourse.masks import make_identity

BF16 = mybir.dt.bfloat16
F8 = mybir.dt.float8e4
F32 = mybir.dt.float32
bf16 = ml_dtypes.bfloat16
e4m3 = ml_dtypes.float8_e4m3fn
DR = mybir.MatmulPerfMode.DoubleRow

B, N, E = 2, 2048, 2048
H, D, G = 16, 128, 4
HKV = H // G
JL = G * D                     # 512 local q-head dims per core
ET = E // 128                  # 16
NT = N // 128                  # 16
CH = N // 512                  # 4
# NOTE: the PE's float8e4 is IEEE-style e4m3 — exponent 1111 is inf/NaN,
# max finite value 240 (not e4m3fn's 448). Scales keep |values| <= ~225.
SX = 32.0                      # x scale (2^5), |x*32| <= ~170
SW = 2048.0                    # Wq/Wk/Wv scale (2^11), |W*2048| <= ~225
SO = 32.0                      # o scale (2^5); |o| <= max|v| ~ 4.3 -> <= 138
SWO = 2048.0                   # Wo scale (2^11)
PSC = SX * SW                  # 2^16, carried by kT/qT/v
OUT_DESCALE = 1.0 / (SO * SWO)          # 2^-17, applied on host
SCALE = 1.0 / (float(np.sqrt(D)) * PSC * PSC)  # exp scale

_cached = {}


def _build(iters=1):
    nc = bacc.Bacc("TRN2", target_bir_lowering=False, debug=False, num_devices=8)

    xhl = nc.dram_tensor("xhl", [E, 2, N], F8, kind="ExternalInput")
    wq = nc.dram_tensor("wq", [E, 2, JL], F8, kind="ExternalInput")
    wk = nc.dram_tensor("wk", [E, 2, D], F8, kind="ExternalInput")
    wv = nc.dram_tensor("wv", [E, 2, D], F8, kind="ExternalInput")
    wo = nc.dram_tensor("wo", [JL, 2, E], F8, kind="ExternalInput")
    out = nc.dram_tensor("out", [N, E], F32, kind="ExternalOutput")

    with tile.TileContext(nc) as tc:
        with (
            tc.tile_pool(name="const", bufs=1) as cpool,
            tc.tile_pool(name="xp", bufs=1) as xpool,
            tc.tile_pool(name="wp", bufs=1) as wpool,
            tc.tile_pool(name="kvp", bufs=1) as kvpool,
            tc.tile_pool(name="qp", bufs=1) as qpool,
            tc.tile_pool(name="pp", bufs=2) as ppool,
            tc.tile_pool(name="op", bufs=4) as opool,
            tc.tile_pool(name="otp", bufs=1) as otpool,
            tc.tile_pool(name="outp", bufs=3) as outpool,
            tc.tile_pool(name="ps1", bufs=2, space="PSUM") as P1,
            tc.tile_pool(name="ps2", bufs=4, space="PSUM") as P2,
        ):
            ident = cpool.tile([128, 128], BF16, tag="ident")
            make_identity(nc, ident[:])

            for _ in range(iters):
                _emit_iter(nc, tc, ident, xpool, wpool, kvpool, qpool, ppool,
                           opool, otpool, outpool, P1, P2,
                           xhl, wq, wk, wv, wo, out)

    nc.compile()
    return nc


def _emit_iter(nc, tc, ident, xpool, wpool, kvpool, qpool, ppool, opool,
               otpool, outpool, P1, P2, xhl, wq, wk, wv, wo, out):
    x_sb = xpool.tile([128, ET, 2, N], F8, tag="x")        # (hi, lo)
    wq_sb = wpool.tile([128, ET, 2, JL], F8, tag="wq")     # (lo, hi)
    wk_sb = wpool.tile([128, ET, 2, D], F8, tag="wk")
    wv_sb = wpool.tile([128, ET, 2, D], F8, tag="wv")
    wo_sb = wpool.tile([128, G, 2, E], F8, tag="wo")
    kT_sb = kvpool.tile([128, N], BF16, tag="kT")
    v_sb = kvpool.tile([128, NT, 130], BF16, tag="v")
    qT_sb = qpool.tile([128, G, N], BF16, tag="qT")
    oT_hl = otpool.tile([128, G, 2, N], F8, tag="oT")      # (hi, lo)

    # --- input DMAs, in consumption order ---
    nc.sync.dma_start(wk_sb[:], wk.rearrange("(a p) t d -> p a t d", p=128))
    nc.scalar.dma_start(wv_sb[:], wv.rearrange("(a p) t d -> p a t d", p=128))
    xr = xhl.rearrange("(a p) t n -> p a t n", p=128)
    for i in range(8):
        eng = nc.sync if i % 2 == 0 else nc.scalar
        eng.dma_start(x_sb[:, 2 * i:2 * i + 2, :, :], xr[:, 2 * i:2 * i + 2, :, :])
    wqr = wq.rearrange("(a p) t j -> p a t j", p=128)
    for i in range(2):
        nc.gpsimd.dma_start(wq_sb[:, 8 * i:8 * i + 8, :, :],
                            wqr[:, 8 * i:8 * i + 8, :, :])
    wor = wo.rearrange("(a p) t e -> p a t e", p=128)
    for jt in range(G):
        nc.gpsimd.dma_start(wo_sb[:, jt, :, :], wor[:, jt, :, :])

    nc.vector.memset(v_sb[:, :, 128:129], PSC)

    # --- refined DR emission helpers ---
    # x stored (hi,lo); weights stored (lo,hi). Per et-pair ep:
    #   T1:    w_hi[2ep:2ep+2] (x) x_hi[2ep:2ep+2]   (one DR)
    #   cross: [w_lo,w_hi][et] (x) [x_hi,x_lo][et]   (one DR per et)
    def kq_dr(ps, w_sb, jsl, nsl, ep, start, stop):
        """k/q-proj: lhsT = weight slices, rhs = x slices."""
        et = 2 * ep
        nc.tensor.matmul(ps, w_sb[:, et:et + 2, 1, jsl], x_sb[:, et:et + 2, 0, nsl],
                         start=start, stop=False, perf_mode=DR)
        nc.tensor.matmul(ps, w_sb[:, et, :, jsl], x_sb[:, et, :, nsl],
                         start=False, stop=False, perf_mode=DR)
        nc.tensor.matmul(ps, w_sb[:, et + 1, :, jsl], x_sb[:, et + 1, :, nsl],
                         start=False, stop=stop, perf_mode=DR)

    def v_dr(ps, nsl, ep, start, stop):
        """v-proj: lhsT = x slices, rhs = wv slices."""
        et = 2 * ep
        nc.tensor.matmul(ps, x_sb[:, et:et + 2, 0, nsl], wv_sb[:, et:et + 2, 1, :],
                         start=start, stop=False, perf_mode=DR)
        nc.tensor.matmul(ps, x_sb[:, et, :, nsl], wv_sb[:, et, :, :],
                         start=False, stop=False, perf_mode=DR)
        nc.tensor.matmul(ps, x_sb[:, et + 1, :, nsl], wv_sb[:, et + 1, :, :],
                         start=False, stop=stop, perf_mode=DR)

    # --- phase 1 ---
    kp = [P1.tile([128, 1024], F32, tag="mm1024", name=f"kp{_i}") for _i in range(2)]
    kps = [kp[_i // 2][:, (_i % 2) * 512:(_i % 2 + 1) * 512] for _i in range(CH)]
    vps = [P2.tile([128, 130], F32, tag="oc", name=f"vps{_i}") for _i in range(4)]
    for ep in range(8):
        for sc in range(CH):
            kq_dr(kps[sc], wk_sb, slice(0, D), slice(sc * 512, (sc + 1) * 512),
                  ep, start=(ep == 0), stop=(ep == 7))
        for st in range(4):
            v_dr(vps[st][:, 0:128], slice(st * 128, (st + 1) * 128),
                 ep, start=(ep == 0), stop=(ep == 7))
    for sc in range(CH):
        nc.vector.tensor_copy(kT_sb[:, sc * 512:(sc + 1) * 512], kps[sc])
    for st in range(4):
        nc.vector.tensor_copy(v_sb[:, st, 0:128], vps[st][:, 0:128])

    def emit_v(st):
        ps = P2.tile([128, 130], F32, tag="oc")
        for ep in range(8):
            v_dr(ps[:, 0:128], slice(st * 128, (st + 1) * 128),
                 ep, start=(ep == 0), stop=(ep == 7))
        nc.vector.tensor_copy(v_sb[:, st, 0:128], ps[:, 0:128])

    def emit_q_pair(q0, q1):
        ps = P1.tile([128, 1024], F32, tag="mm1024")
        for half, (g, ncg) in enumerate((q0, q1)):
            sl = ps[:, half * 512:(half + 1) * 512]
            for ep in range(8):
                kq_dr(sl, wq_sb, slice(g * 128, (g + 1) * 128),
                      slice(ncg * 512, (ncg + 1) * 512),
                      ep, start=(ep == 0), stop=(ep == 7))
            nc.vector.tensor_copy(qT_sb[:, g, ncg * 512:(ncg + 1) * 512], sl)

    qlist = [(g, ncg) for g in range(G) for ncg in range(CH)]
    vq = list(range(4, NT))

    # --- phase 2 + 3, pipelined per chunk of 512 n-columns ---
    def emit_score_pair(g, c, p_t, sp):
        ps = P1.tile([128, 1024], F32, tag="mm1024")
        for half in range(2):
            st = 2 * sp + half
            nc.tensor.matmul(
                ps[:, half * 512:(half + 1) * 512],
                kT_sb[:, st * 128:(st + 1) * 128],
                qT_sb[:, g, c * 512:(c + 1) * 512],
                start=True, stop=True,
            )
        nc.scalar.activation(
            p_t[:, 2 * sp * 512:(2 * sp + 2) * 512], ps[:],
            mybir.ActivationFunctionType.Exp, scale=SCALE,
        )

    def emit_o_group(g, c, p_t, t):
        pso = P2.tile([128, 130], F32, tag="oc")
        for st in range(NT):
            nc.tensor.matmul(
                pso[:, 0:129], p_t[:, st * 512 + t * 128: st * 512 + (t + 1) * 128],
                v_sb[:, st, 0:129],
                start=(st == 0), stop=(st == NT - 1),
            )
        rc = opool.tile([128, 1], F32, tag="recip")
        nc.vector.reciprocal(rc[:], pso[:, 128:129])
        o_n = opool.tile([128, 128], BF16, tag="o_n")
        nc.vector.tensor_scalar_mul(o_n[:], pso[:, 0:128], rc[:])
        pst = P2.tile([128, 128], BF16, tag="oc")
        nc.tensor.transpose(pst[:], o_n[:], ident[:])
        osl = slice(c * 512 + t * 128, c * 512 + (t + 1) * 128)
        nc.vector.tensor_scalar_mul(oT_hl[:, g, 0, osl], pst[:], SO)
        nc.vector.scalar_tensor_tensor(
            oT_hl[:, g, 1, osl], pst[:], SO, oT_hl[:, g, 0, osl],
            mybir.AluOpType.mult, mybir.AluOpType.subtract,
        )

    def emit_out_nt(nt):
        nsl = slice(nt * 128, (nt + 1) * 128)
        for half in range(2):
            stage = outpool.tile([128, 1024], F32, tag="out")
            ps = P1.tile([128, 1024], F32, tag="mm1024")
            for e2 in range(2):
                ec = half * 2 + e2
                esl = slice(ec * 512, (ec + 1) * 512)
                sl = ps[:, e2 * 512:(e2 + 1) * 512]
                for gp in range(2):
                    nc.tensor.matmul(
                        sl, oT_hl[:, 2 * gp:2 * gp + 2, 0, nsl],
                        wo_sb[:, 2 * gp:2 * gp + 2, 1, esl],
                        start=(gp == 0), stop=False, perf_mode=DR,
                    )
                for g in range(G):
                    nc.tensor.matmul(
                        sl, oT_hl[:, g, :, nsl], wo_sb[:, g, :, esl],
                        start=False, stop=(g == G - 1), perf_mode=DR,
                    )
            nc.vector.tensor_copy(stage[:], ps[:])
            eng = nc.sync if (nt + half) % 2 == 0 else nc.scalar
            eng.dma_start(
                out[nt * 128:(nt + 1) * 128, half * 1024:(half + 1) * 1024],
                stage[:],
            )

    def emit_out(c):
        for nt in range(4 * c, 4 * c + 4):
            emit_out_nt(nt)

    for i in range(8):
        emit_q_pair(qlist[2 * i], qlist[2 * i + 1])
        for _ in range(2):
            if vq:
                emit_v(vq.pop(0))

    chunks = [(c, g) for c in range(CH) for g in range(G)]
    prev = None
    for i, (c, g) in enumerate(chunks):
        p_t = ppool.tile([128, NT * 512], BF16, tag="p", name=f"p{i}")
        for sub in range(4):
            emit_score_pair(g, c, p_t, 2 * sub)
            emit_score_pair(g, c, p_t, 2 * sub + 1)
            if prev is not None:
                emit_o_group(prev[0], prev[1], prev[2], sub)
        if i > 0 and i % 4 == 0:
            emit_out(i // 4 - 1)
        prev = (g, c, p_t)
    for sub in range(4):
        emit_o_group(prev[0], prev[1], prev[2], sub)
        emit_out_nt(4 * (CH - 1) + sub)


def get_nc(iters=1):
    key = ("nc", iters)
    if key not in _cached:
        _cached[key] = _build(iters)
    return _cached[key]


def _split_hl(a, scale, lo_first):
    """e4m3 hi/lo split at shared power-of-2 scale, stacked on axis 1.

    Clips to +-240 (IEEE e4m3 max finite) so no encoding maps to the
    hardware's inf/NaN exponent.
    """
    s = np.clip((a * scale).astype(np.float32), -240.0, 240.0)
    hi = s.astype(e4m3)
    lo = np.clip(s - hi.astype(np.float32), -240.0, 240.0).astype(e4m3)
    pair = (lo, hi) if lo_first else (hi, lo)
    return np.ascontiguousarray(np.stack(pair, axis=1))


def make_in_maps(x, Wq, Wk, Wv, Wo):
    """Per-core host-side sharding. Core c -> (b=c//4, gk=c%4)."""
    x = np.asarray(x, np.float32)
    Wq = np.asarray(Wq, np.float32)
    Wk = np.asarray(Wk, np.float32)
    Wv = np.asarray(Wv, np.float32)
    Wo = np.asarray(Wo, np.float32)
    xhl = [_split_hl(x[b].T, SX, lo_first=False) for b in range(B)]
    wq_s = [_split_hl(Wq[gk * JL:(gk + 1) * JL, :].T, SW, lo_first=True)
            for gk in range(HKV)]
    wk_s = [_split_hl(Wk[gk * D:(gk + 1) * D, :].T, SW, lo_first=True)
            for gk in range(HKV)]
    wv_s = [_split_hl(Wv[gk * D:(gk + 1) * D, :].T, SW, lo_first=True)
            for gk in range(HKV)]
    wo_s = [_split_hl(Wo[:, gk * JL:(gk + 1) * JL].T, SWO, lo_first=True)
            for gk in range(HKV)]
    in_maps = []
    for c in range(8):
        b, gk = c // 4, c % 4
        in_maps.append({
            "xhl": xhl[b], "wq": wq_s[gk], "wk": wk_s[gk],
            "wv": wv_s[gk], "wo": wo_s[gk],
        })
    return in_maps


def kernel(x, Wq, Wk, Wv, Wo):
    nc = get_nc()
    in_maps = make_in_maps(x, Wq, Wk, Wv, Wo)
    res = run_bass_kernel_spmd(nc, in_maps, core_ids=list(range(8)))
    out = np.empty((B, N, E), np.float32)
    for b in range(B):
        acc = res.results[b * 4]["out"]
        for gk in range(1, HKV):
            acc = acc + res.results[b * 4 + gk]["out"]
        out[b] = acc * OUT_DESCALE
    return out
